# revision 1
# baseline (speedup 1.0000x reference)
"""MEX (log-mean-exp) 3x3 pooling kernel for Trainium2, 8-core data-parallel.

Math: out[n,i,h,w] = log( (1/K) * sum_{c,kh,kw} exp(x[n,c,h+kh-1,w+kw-1] + o[i,c,kh,kw]) )
with zero-padded x OOB (contributing exp(0+o) = exp(o)) and K = 32*3*3 = 288.

Factorization used on-device (EPS=1, no max-subtraction needed in f32 range):
    out = log( (1/K) * conv3x3( exp(xpad), exp(o) ) )
where exp(xpad) has 1.0 at padding (= exp(0)).

Per-core mapping (one image per core):
  - SBUF holds 3 row-shifted padded planes of exp(x) in bf16, partitions
    (kh, c) = 96, so one matmul contracts (c, kh) = 96 and the kw shift is a
    free-dim offset accumulated over 3 matmuls into PSUM (f32).
  - Output strip q (4 rows) lands at psum partitions 32q..32q+31 via
    tile_position (from base partitions), so a full [128, 512] bank = 16
    output rows gets ONE wide Ln activation pass (scale=1/288) to SBUF f32.
"""

import numpy as np

import concourse.bacc as bacc
import concourse.tile as tile
import concourse.mybir as mybir
from concourse.bass_utils import run_bass_kernel_spmd

F32 = mybir.dt.float32
BF16 = mybir.dt.bfloat16
AF = mybir.ActivationFunctionType

N, C, H, W = 8, 32, 128, 128
I = 32
K = C * 3 * 3          # 288
BR = 16                # output rows per band
BANDS = H // BR        # 8
WP = 132               # padded+aligned plane width (cols 0..129 used, 130/131 slack)


def _build(repeats: int = 1):
    nc = bacc.Bacc("TRN2", target_bir_lowering=False, debug=False)
    x = nc.dram_tensor("x", [C, H, W], F32, kind="ExternalInput").ap()
    off = nc.dram_tensor("offsets", [1, I, C, 3, 3], F32, kind="ExternalInput").ap()
    out = nc.dram_tensor("out", [I, H, W], F32, kind="ExternalOutput").ap()

    # out rows h = 32t + 16u + 4q + r
    out_r = out.rearrange("i (t u q r) w -> i t u q r w", t=4, u=2, q=4, r=4)

    with tile.TileContext(nc) as tc:
        with (
            tc.tile_pool(name="const", bufs=1) as constp,
            tc.tile_pool(name="xg", bufs=3) as xgp,
            tc.tile_pool(name="e3", bufs=3) as e3p,
            tc.tile_pool(name="ps", bufs=4, space="PSUM") as psp,
            tc.tile_pool(name="ob", bufs=2) as obp,
        ):
            # weights: W[(kh,c), kw, i] = exp(o[i,c,kh,kw]) in bf16
            wf = constp.tile([96, 3, I], F32)
            wb = constp.tile([96, 3, I], BF16)
            off_t = off[0].transpose((2, 3, 1, 0))  # [kh, kw, c, i]
            for kh in range(3):
                for kw in range(3):
                    nc.sync.dma_start(
                        wf[32 * kh : 32 * kh + 32, kw], off_t[kh, kw]
                    )
            nc.scalar.activation(wb[:], wf[:], AF.Exp)

            for _rep in range(repeats):
                _emit_body(nc, tc, x, out_r, wb, xgp, e3p, psp, obp)
    nc.compile()
    return nc


def _emit_body(nc, tc, x, out_r, wb, xgp, e3p, psp, obp):
    if True:
        if True:
            for pair in range(BANDS // 2):
                ob = obp.tile([128, 2, 4, W], F32)
                for u in range(2):
                    b = pair * 2 + u
                    # xg[(g,c), s, wp] = xpad[c, BR*b + s + g, wp] ; xpad = zero-pad(x)
                    xg = xgp.tile([96, BR, WP], F32)
                    nc.vector.memset(xg[:, :, 0:1], 0.0)
                    nc.vector.memset(xg[:, :, 129:132], 0.0)
                    if b == 0:
                        nc.vector.memset(xg[0:32, 0:1, :], 0.0)
                    if b == BANDS - 1:
                        nc.vector.memset(xg[64:96, BR - 1 : BR, :], 0.0)
                    for g in range(3):
                        r0 = BR * b + g - 1        # x row held in slot 0
                        s0 = max(0, -r0)           # first valid slot
                        r1 = min(H, r0 + BR)       # exclusive end x row
                        n = r1 - (r0 + s0)
                        nc.sync.dma_start(
                            xg[32 * g : 32 * g + 32, s0 : s0 + n, 1 : 1 + W],
                            x[:, r0 + s0 : r1, :],
                        )
                    e3 = e3p.tile([96, BR, WP], BF16)
                    nc.scalar.activation(e3[:], xg[:], AF.Exp)

                    ps = psp.tile([128, 4, W], F32)
                    for q in range(4):
                        for kw in range(3):
                            nc.tensor.matmul(
                                ps[32 * q : 32 * q + 32, :, :],
                                wb[:, kw, :],
                                e3[:, 4 * q : 4 * q + 4, kw : kw + W],
                                start=(kw == 0),
                                stop=(kw == 2),
                                tile_position=(0, 32 * q),
                            )
                    nc.scalar.activation(ob[:, u], ps[:], AF.Ln, scale=1.0 / K)
                for q in range(4):
                    nc.sync.dma_start(
                        out_r[:, pair, :, q], ob[32 * q : 32 * q + 32]
                    )


_NC = None


def _get_nc():
    global _NC
    if _NC is None:
        _NC = _build()
    return _NC


def kernel(x: np.ndarray, offsets: np.ndarray) -> np.ndarray:
    x = np.ascontiguousarray(x, dtype=np.float32)
    offsets = np.ascontiguousarray(offsets, dtype=np.float32)
    nc = _get_nc()
    in_maps = [
        {"x": np.ascontiguousarray(x[i]), "offsets": offsets} for i in range(N)
    ]
    res = run_bass_kernel_spmd(nc, in_maps, list(range(N))).results
    return np.stack([res[i]["out"] for i in range(N)], axis=0)



# revision 2
# speedup vs baseline: 106.2389x; 106.2389x over previous
"""MEX (log-mean-exp) 3x3 pooling kernel for Trainium2, 8-core data-parallel. v2.

Math: out[n,i,h,w] = log( (1/K) * sum_{c,kh,kw} exp(x[n,c,h+kh-1,w+kw-1] + o[i,c,kh,kw]) )
with zero-padded x OOB (contributing exp(0+o) = exp(o)) and K = 32*3*3 = 288.

Factorization on-device (EPS=1, f32 range needs no max-shift):
    out = log( (1/K) * conv3x3( exp(xpad), exp(o) ) )

v2 layout (per core, one image):
  - SBUF partition p = 32*q + c, q = h//32 (row quarter), c = channel.
    Each partition holds the FLAT row-major slab x[c, 32q-1 : 32(q+1)+1, :]
    plus 1-elem halo: xr[p, t] = x.flat[c, 4096q - 129 + t], t in [0, 4354).
    -> input is 4 DMAs with ~17 KiB fully-contiguous descriptors per
    partition (vs 512 B strided lines), one Exp pass over [128, 4354].
  - conv term (kh, kw) is a flat offset 128*kh + kw into the slab, so each
    512-position wave is 9 accumulating matmuls (contraction = c = 32).
    The 4 quarters run on diagonal PE tiles (32q, 32q) concurrently.
  - Flat addressing wraps at W edges: w=0 (kw=0) and w=127 (kw=2) read the
    neighbouring row. Those 2 columns are recomputed exactly: 6 interior
    terms per side as free-dim-32 matmuls + the pad term sum_c,kh exp(o)
    folded into the Ln bias; results overwrite the border columns in SBUF
    before the output DMA.
  - Output: psum [128, 512] per wave -> one wide Ln (scale=1/288) -> two
    [128, 4x512] DMAs (8 KiB contiguous per partition).
"""

import numpy as np

import concourse.bacc as bacc
import concourse.tile as tile
import concourse.mybir as mybir
from concourse.bass_utils import run_bass_kernel_spmd

F32 = mybir.dt.float32
BF16 = mybir.dt.bfloat16
AF = mybir.ActivationFunctionType
AX = mybir.AxisListType

N, C, H, W = 8, 32, 128, 128
I = 32
K = C * 3 * 3          # 288
FL = 32 * W            # 4096 flat positions per quarter
PAD = W + 1            # 129: one row + one col of halo
TLEN = PAD + FL + PAD  # 4354
WV = 512               # positions per wave (one PSUM bank)
NW = FL // WV          # 8 waves


def _build(repeats: int = 1, loop_iters: int | None = None):
    nc = bacc.Bacc("TRN2", target_bir_lowering=False, debug=False)
    x = nc.dram_tensor("x", [C, H, W], F32, kind="ExternalInput").ap()
    off = nc.dram_tensor("offsets", [1, I, C, 3, 3], F32, kind="ExternalInput").ap()
    out = nc.dram_tensor("out", [I, H, W], F32, kind="ExternalOutput").ap()

    x_f = x.rearrange("c h w -> c (h w)")                       # [32, 16384]
    out_r = out.rearrange("i (q k r) w -> q i k (r w)", q=4, k=NW, r=4)

    with tile.TileContext(nc) as tc:
        with (
            tc.tile_pool(name="const", bufs=1) as constp,
            tc.tile_pool(name="xr", bufs=3) as xrp,
            tc.tile_pool(name="ex", bufs=3) as exp_,
            tc.tile_pool(name="ob", bufs=2) as obp,
            tc.tile_pool(name="obb", bufs=2) as obbp,
            tc.tile_pool(name="ps", bufs=3, space="PSUM") as psp,
            tc.tile_pool(name="psb", bufs=2, space="PSUM") as psbp,
        ):
            # ---- weights: wT[(q,c), t=(3kh+kw), i] = exp(o[i,c,kh,kw]), bf16 ----
            offs = constp.tile([128, C, 3, 3], F32)
            for q in range(4):
                nc.sync.dma_start(offs[32 * q : 32 * q + 32], off[0])
            offsT = constp.tile([128, 9, I], F32)
            for kh in range(3):
                for kw in range(3):
                    nc.vector.transpose(
                        offsT[:, 3 * kh + kw, :], offs[:, :, kh, kw]
                    )
            wT = constp.tile([128, 9, I], BF16)
            nc.scalar.activation(wT[:], offsT[:], AF.Exp)
            # pad-term bias: pb[(q,i), s] = (1/K) * sum_{c,kh} exp(o[i,c,kh, 0 or 2])
            offe = constp.tile([128, C, 3, 3], F32)
            nc.scalar.activation(offe[:], offs[:], AF.Exp)
            pb = constp.tile([128, 2], F32)
            nc.vector.reduce_sum(pb[:, 0:1], offe[:, :, :, 0], axis=AX.XY)
            nc.vector.reduce_sum(pb[:, 1:2], offe[:, :, :, 2], axis=AX.XY)
            pbs = constp.tile([128, 2], F32)
            nc.vector.tensor_scalar_mul(pbs[:], pb[:], 1.0 / K)

            if loop_iters is None:
                for _rep in range(repeats):
                    _emit_body(
                        nc, x_f, out_r, wT, pbs, xrp, exp_, obp, obbp, psp, psbp
                    )
            else:
                with tc.For_i(0, loop_iters):
                    for _rep in range(repeats):
                        _emit_body(
                            nc, x_f, out_r, wT, pbs, xrp, exp_, obp, obbp, psp, psbp
                        )
    nc.compile()
    return nc


def _emit_body(nc, x_f, out_r, wT, pbs, xrp, exp_, obp, obbp, psp, psbp):
    # ---- load flat slabs + halo, one Exp pass ----
    xr = xrp.tile([128, TLEN], F32)
    nc.vector.memset(xr[0:32, 0:PAD], 0.0)
    nc.vector.memset(xr[96:128, TLEN - PAD : TLEN], 0.0)
    for q in range(4):
        s0 = FL * q - PAD
        t0 = max(0, -s0)
        s1 = min(H * W, s0 + TLEN)
        nc.sync.dma_start(
            xr[32 * q : 32 * q + 32, t0 : s1 - s0], x_f[:, s0 + t0 : s1]
        )
    ex = exp_.tile([128, TLEN], BF16)
    nc.scalar.activation(ex[:], xr[:], AF.Exp)

    # ---- border columns (w=0 and w=127), exact recompute ----
    # psB[(q,i), s*32 + r] = sum of the 6 in-bounds terms for output row
    # h = 32q + r at w=0 (s=0) / w=127 (s=1).
    psB = psbp.tile([128, 2, 32], F32)
    n_t = 0
    for s, kws in ((0, (1, 2)), (1, (0, 1))):
        col = (0, 127)[s]
        for kh in range(3):
            for kw in kws:
                base = 128 * kh + (kw if s else 0)
                rv = ex[:, base : base + FL].rearrange(
                    "p (r w) -> p r w", w=W
                )[:, :, col if s else kw]
                for q in range(4):
                    nc.tensor.matmul(
                        psB[32 * q : 32 * q + 32, s, :],
                        wT[32 * q : 32 * q + 32, 3 * kh + kw, :],
                        rv[32 * q : 32 * q + 32],
                        start=(n_t == 0),
                        stop=(n_t == 5),
                        tile_position=(32 * q, 32 * q),
                        skip_group_check=True,
                    )
                n_t = (n_t + 1) % 6
    obB = obbp.tile([128, 2, 32], F32)
    nc.scalar.activation(obB[:, 0], psB[:, 0], AF.Ln, scale=1.0 / K, bias=pbs[:, 0:1])
    nc.scalar.activation(obB[:, 1], psB[:, 1], AF.Ln, scale=1.0 / K, bias=pbs[:, 1:2])

    # ---- main waves, processed in pairs sharing a 2-bank psum tile ----
    ob = obp.tile([128, NW, WV], F32)
    for kp in range(NW // 2):
        ps = psp.tile([128, 2, WV], F32)
        for half in range(2):
            k = 2 * kp + half
            for t in range(9):
                kh, kw = divmod(t, 3)
                o = WV * k + 128 * kh + kw
                for q in range(4):
                    nc.tensor.matmul(
                        ps[32 * q : 32 * q + 32, half, :],
                        wT[32 * q : 32 * q + 32, t, :],
                        ex[32 * q : 32 * q + 32, o : o + WV],
                        start=(t == 0),
                        stop=(t == 8),
                        tile_position=(32 * q, 32 * q),
                        skip_group_check=True,
                    )
        nc.scalar.activation(ob[:, 2 * kp : 2 * kp + 2], ps[:], AF.Ln, scale=1.0 / K)

        k = 2 * kp + 1
        if k == NW // 2 - 1 or k == NW - 1:
            lo, hi = (0, NW // 2) if k == NW // 2 - 1 else (NW // 2, NW)
            # overwrite border columns, then ship the half
            obv = ob[:, lo:hi].rearrange("p k (r w) -> p k r w", w=W)
            obBv = obB.rearrange("p s (k r) -> p s k r", r=4)
            nc.vector.tensor_copy(obv[:, :, :, 0], obBv[:, 0, lo:hi])
            nc.vector.tensor_copy(obv[:, :, :, 127], obBv[:, 1, lo:hi])
            for q in range(4):
                nc.sync.dma_start(
                    out_r[q][:, lo:hi], ob[32 * q : 32 * q + 32, lo:hi]
                )


_NC = None


def _get_nc():
    global _NC
    if _NC is None:
        _NC = _build()
    return _NC


def kernel(x: np.ndarray, offsets: np.ndarray) -> np.ndarray:
    x = np.ascontiguousarray(x, dtype=np.float32)
    offsets = np.ascontiguousarray(offsets, dtype=np.float32)
    nc = _get_nc()
    in_maps = [
        {"x": np.ascontiguousarray(x[i]), "offsets": offsets} for i in range(N)
    ]
    res = run_bass_kernel_spmd(nc, in_maps, list(range(N))).results
    return np.stack([res[i]["out"] for i in range(N)], axis=0)


# revision 4
# speedup vs baseline: 108.6674x; 1.0229x over previous
"""MEX (log-mean-exp) 3x3 pooling kernel for Trainium2, 8-core data-parallel.

Math: out[n,i,h,w] = log( (1/K) * sum_{c,kh,kw} exp(x[n,c,h+kh-1,w+kw-1] + o[i,c,kh,kw]) )
with zero-padded x OOB (contributing exp(0+o) = exp(o)) and K = 32*3*3 = 288.

Factorization on-device (EPS=1, f32 range needs no max-shift):
    out = log( (1/K) * conv3x3( exp(xpad), exp(o) ) )

Layout (per core, one image):
  - SBUF partition p = 32*q + c, q = h//32 (row quarter), c = channel.
    Each partition holds the FLAT row-major slab x[c, 32q-1 : 32(q+1)+1, :]
    plus 1-elem halo: xr[p, t] = x.flat[c, 4096q - 129 + t], t in [0, 4354).
    -> input is 4 DMAs with ~17 KiB fully-contiguous descriptors per
    partition (vs 512 B strided lines), one Exp pass over [128, 4354].
  - conv term (kh, kw) is a flat offset 128*kh + kw into the slab, so each
    512-position wave is 9 accumulating matmuls (contraction = c = 32).
    The 4 quarters run on diagonal PE tiles (32q, 32q) concurrently.
  - Flat addressing wraps at W edges: w=0 (kw=0) and w=127 (kw=2) read the
    neighbouring row. Those 2 columns are recomputed exactly: 6 interior
    terms per side as free-dim-32 matmuls + the pad term sum_c,kh exp(o)
    folded into the Ln bias; results overwrite the border columns in SBUF
    before the output DMA.
  - Output: psum [128, 512] per wave -> one wide Ln (scale=1/288) -> two
    [128, 4x512] DMAs (8 KiB contiguous per partition).
"""

import numpy as np

import concourse.bacc as bacc
import concourse.tile as tile
import concourse.mybir as mybir
from concourse.bass_utils import run_bass_kernel_spmd

F32 = mybir.dt.float32
BF16 = mybir.dt.bfloat16
AF = mybir.ActivationFunctionType
AX = mybir.AxisListType

N, C, H, W = 8, 32, 128, 128
I = 32
K = C * 3 * 3          # 288
FL = 32 * W            # 4096 flat positions per quarter
PAD = W + 1            # 129: one row + one col of halo
TLEN = PAD + FL + PAD  # 4354
WV = 512               # positions per wave (one PSUM bank)
NW = FL // WV          # 8 waves


def _build(repeats: int = 1, loop_iters: int | None = None):
    nc = bacc.Bacc("TRN2", target_bir_lowering=False, debug=False)
    x = nc.dram_tensor("x", [C, H, W], F32, kind="ExternalInput").ap()
    off = nc.dram_tensor("offsets", [1, I, C, 3, 3], F32, kind="ExternalInput").ap()
    out = nc.dram_tensor("out", [I, H, W], F32, kind="ExternalOutput").ap()

    x_f = x.rearrange("c h w -> c (h w)")                       # [32, 16384]
    out_r = out.rearrange("i (q k r) w -> q i k (r w)", q=4, k=NW, r=4)

    with tile.TileContext(nc) as tc:
        with (
            tc.tile_pool(name="const", bufs=1) as constp,
            tc.tile_pool(name="xr", bufs=3) as xrp,
            tc.tile_pool(name="ex", bufs=3) as exp_,
            tc.tile_pool(name="ob", bufs=2) as obp,
            tc.tile_pool(name="obb", bufs=2) as obbp,
            tc.tile_pool(name="ps", bufs=3, space="PSUM") as psp,
            tc.tile_pool(name="psb", bufs=2, space="PSUM") as psbp,
        ):
            # ---- weights: wT[(q,c), t=(3kh+kw), i] = exp(o[i,c,kh,kw]), bf16 ----
            offs = constp.tile([128, C, 3, 3], F32)
            for q in range(4):
                nc.sync.dma_start(offs[32 * q : 32 * q + 32], off[0])
            offsT = constp.tile([128, 9, I], F32)
            for kh in range(3):
                for kw in range(3):
                    nc.vector.transpose(
                        offsT[:, 3 * kh + kw, :], offs[:, :, kh, kw]
                    )
            wT = constp.tile([128, 9, I], BF16)
            nc.scalar.activation(wT[:], offsT[:], AF.Exp)
            # pad-term bias: pb[(q,i), s] = (1/K) * sum_{c,kh} exp(o[i,c,kh, 0 or 2])
            offe = constp.tile([128, C, 3, 3], F32)
            nc.scalar.activation(offe[:], offs[:], AF.Exp)
            pb = constp.tile([128, 2], F32)
            nc.vector.reduce_sum(pb[:, 0:1], offe[:, :, :, 0], axis=AX.XY)
            nc.vector.reduce_sum(pb[:, 1:2], offe[:, :, :, 2], axis=AX.XY)
            pbs = constp.tile([128, 2], F32)
            nc.vector.tensor_scalar_mul(pbs[:], pb[:], 1.0 / K)

            if loop_iters is None:
                for _rep in range(repeats):
                    _emit_body(
                        nc, x_f, out_r, wT, pbs, xrp, exp_, obp, obbp, psp, psbp
                    )
            else:
                with tc.For_i(0, loop_iters):
                    for _rep in range(repeats):
                        _emit_body(
                            nc, x_f, out_r, wT, pbs, xrp, exp_, obp, obbp, psp, psbp
                        )
    nc.compile()
    return nc


def _emit_body(nc, x_f, out_r, wT, pbs, xrp, exp_, obp, obbp, psp, psbp):
    # ---- load flat slabs + halo, one Exp pass ----
    xr = xrp.tile([128, TLEN], F32)
    nc.vector.memset(xr[0:32, 0:PAD], 0.0)
    nc.vector.memset(xr[96:128, TLEN - PAD : TLEN], 0.0)
    for q in range(4):
        s0 = FL * q - PAD
        t0 = max(0, -s0)
        s1 = min(H * W, s0 + TLEN)
        nc.sync.dma_start(
            xr[32 * q : 32 * q + 32, t0 : s1 - s0], x_f[:, s0 + t0 : s1]
        )
    ex = exp_.tile([128, TLEN], BF16)
    nc.scalar.activation(ex[:], xr[:], AF.Exp)

    # ---- border columns (w=0 and w=127), exact recompute ----
    # psB[(q,i), s*32 + r] = sum of the 6 in-bounds terms for output row
    # h = 32q + r at w=0 (s=0) / w=127 (s=1).
    psB = psbp.tile([128, 2, 32], F32)
    n_t = 0
    for s, kws in ((0, (1, 2)), (1, (0, 1))):
        col = (0, 127)[s]
        for kh in range(3):
            for kw in kws:
                base = 128 * kh + (kw if s else 0)
                rv = ex[:, base : base + FL].rearrange(
                    "p (r w) -> p r w", w=W
                )[:, :, col if s else kw]
                for q in range(4):
                    nc.tensor.matmul(
                        psB[32 * q : 32 * q + 32, s, :],
                        wT[32 * q : 32 * q + 32, 3 * kh + kw, :],
                        rv[32 * q : 32 * q + 32],
                        start=(n_t == 0),
                        stop=(n_t == 5),
                        tile_position=(32 * q, 32 * q),
                        skip_group_check=True,
                    )
                n_t = (n_t + 1) % 6
    obB = obbp.tile([128, 2, 32], F32)
    nc.scalar.activation(obB[:, 0], psB[:, 0], AF.Ln, scale=1.0 / K, bias=pbs[:, 0:1])
    nc.scalar.activation(obB[:, 1], psB[:, 1], AF.Ln, scale=1.0 / K, bias=pbs[:, 1:2])

    # ---- main waves, processed in pairs sharing a 2-bank psum tile ----
    ob = obp.tile([128, NW, WV], F32)
    for kp in range(NW // 2):
        ps = psp.tile([128, 2, WV], F32)
        for half in range(2):
            k = 2 * kp + half
            for t in range(9):
                kh, kw = divmod(t, 3)
                o = WV * k + 128 * kh + kw
                for q in range(4):
                    nc.tensor.matmul(
                        ps[32 * q : 32 * q + 32, half, :],
                        wT[32 * q : 32 * q + 32, t, :],
                        ex[32 * q : 32 * q + 32, o : o + WV],
                        start=(t == 0),
                        stop=(t == 8),
                        tile_position=(32 * q, 32 * q),
                        skip_group_check=True,
                    )
        nc.scalar.activation(ob[:, 2 * kp : 2 * kp + 2], ps[:], AF.Ln, scale=1.0 / K)

        k = 2 * kp + 1
        if k == NW // 2 - 1 or k == NW - 1:
            lo, hi = (0, NW // 2) if k == NW // 2 - 1 else (NW // 2, NW)
            # overwrite border columns, then ship the half
            obv = ob[:, lo:hi].rearrange("p k (r w) -> p k r w", w=W)
            obBv = obB.rearrange("p s (k r) -> p s k r", r=4)
            nc.vector.tensor_copy(obv[:, :, :, 0], obBv[:, 0, lo:hi])
            nc.vector.tensor_copy(obv[:, :, :, 127], obBv[:, 1, lo:hi])
            for q in range(4):
                nc.sync.dma_start(
                    out_r[q][:, lo:hi], ob[32 * q : 32 * q + 32, lo:hi]
                )


_NC = None


def _get_nc():
    global _NC
    if _NC is None:
        _NC = _build()
    return _NC


def kernel(x: np.ndarray, offsets: np.ndarray) -> np.ndarray:
    x = np.ascontiguousarray(x, dtype=np.float32)
    offsets = np.ascontiguousarray(offsets, dtype=np.float32)
    nc = _get_nc()
    in_maps = [
        {"x": np.ascontiguousarray(x[i]), "offsets": offsets} for i in range(N)
    ]
    res = run_bass_kernel_spmd(nc, in_maps, list(range(N))).results
    return np.stack([res[i]["out"] for i in range(N)], axis=0)
